# revision 104
# baseline (speedup 1.0000x reference)
"""ByteContextEncoder Trainium2 kernel (v2).

8-core SPMD: core c = (batch row c//2, sequence half c%2), TL=1024 local
tokens. Key structural points vs v1:

- Layer-0 K/V come from a full-row embedding computed locally on every
  core (embedding is cheap) -> no layer-0 collective at all.
- Layer-1 h-exchange is split into two half-row AllGathers issued as the
  FFN finishes each 4-tile group, hiding the collective latency.
- RoPE is applied in token-major layout (free-dim rotate-half with
  stride-0-broadcast cos/sin tables) so q/k need a single projection
  matmul each; transposes to D-major go through the DMA xbar
  (dma_start_transpose), not the PE.
- Attention (Act-bound: exp) and FFN (PE-bound) of each layer are
  interleaved per 4-tile group so the scalar engine's exp runs under the
  FFN matmuls.
- The boundary-segment pair exchange is launched right after the last
  boundary tile's final-norm, hidden under segment pooling.

Host only builds integer-derived index/one-hot/rope tables and casts
weights to bf16.
"""

import math

import numpy as np
import ml_dtypes

import concourse.bass as bass
import concourse.mybir as mybir
import concourse.tile as tile

BF16 = mybir.dt.bfloat16
F32 = mybir.dt.float32
AX = mybir.AxisListType
ALU = mybir.AluOpType
ACT = mybir.ActivationFunctionType

# model dims (hardcoded per problem spec)
B, T, D, H, L = 4, 2048, 512, 8, 2
FF = 4 * D
HD = D // H
EPS = 1e-6
ALPHA = 0.5

N_CORES = 8

TL = T // 2          # tokens per core
P = 128
NT = TL // P         # 8 local token tiles
KT2 = T // P         # 16 full-row token tiles
DC = D // P          # 4 D chunks
FFC = FF // P        # 16 FF chunks
SEG = 256            # padded segments per core
SC = SEG // P        # 3 segment chunks
QBW = 256            # q-block width for attention
NQB = TL // QBW      # 4 q blocks
NF = HD // 2         # 32 rope frequencies

_SEP = b" \t\n\r.,;:!?()[]{}\"'" + b"+-*/=<>|&^~%@#$\\"
SEP_TABLE = np.zeros(256, dtype=bool)
SEP_TABLE[list(_SEP)] = True


def split_multiwait_drains(nc, max_waits=1):
    """This container's walrus can't encode >1 sync-wait on an instruction;
    hoist extra waits onto single-wait NoOps just before it (same engine, so
    sequencer order preserves the wait-before-execute semantics)."""
    n_patched = 0
    for f in nc.m.functions:
        for bb in f.blocks:
            new_list = []
            changed = False
            for ins in bb.instructions:
                si = ins.sync_info
                if si is not None and si.on_wait and len(si.on_wait) > max_waits:
                    for k, w in enumerate(si.on_wait):
                        nop = mybir.InstNoOp(name=f"{ins.name}-w{k}", ins=[], outs=[])
                        nop.engine = ins.engine
                        nop.sync_info = mybir.SyncInfo(on_wait=[w], on_update=[])
                        new_list.append(nop)
                    ins.sync_info = mybir.SyncInfo(
                        on_wait=[], on_update=list(si.on_update)
                    )
                    changed = True
                    n_patched += 1
                new_list.append(ins)
            if changed:
                bb.instructions = new_list
    return n_patched


def strip_transpose_cc_waits(nc):
    """The tile framework serializes every DMA xbar transpose against all
    prior collectives (shared-resource guard). Our transposes never touch
    the collective buffers (deps flow through separate DMA sems), so drop
    those waits; they otherwise stall the FFN pipeline ~30us per exchange."""
    n = 0
    for f in nc.m.functions:
        for bb in f.blocks:
            for ins in bb.instructions:
                if not isinstance(ins, mybir.InstDmaTransposeAnt):
                    continue
                si = ins.sync_info
                if si is None or not si.on_wait:
                    continue
                kept = [
                    w
                    for w in si.on_wait
                    if not (w.ant_name or "").startswith("Collectives")
                ]
                if len(kept) != len(si.on_wait):
                    ins.sync_info = mybir.SyncInfo(
                        on_wait=kept, on_update=list(si.on_update)
                    )
                    n += 1
    return n


def _rep_ap(sl, reps):
    """[P, NF] AP -> [P, reps, NF] AP with stride-0 middle axis."""
    return bass.AP(
        tensor=sl.tensor, offset=sl.offset, ap=[list(sl.ap[0]), [0, reps], list(sl.ap[1])]
    )


def build_program(debug=(), patch=True, stage=5, for_sim=False, apply_fw=False):
    nc = bass.Bass(num_devices=N_CORES)

    # ---------------- DRAM inputs ----------------
    # all small per-partition tables ride in two packed blobs (1 DMA each):
    # blob16: cosr | sinr | cosl | sinl | og | otg | obind | bsel
    # blob32: normt | icnt
    B16W = KT2 * NF * 2 + NT * NF * 2 + NT * SEG + SC * TL + TL + 2
    B32W = NT + SC
    d_blob16 = nc.dram_tensor("blob16", [P, B16W], BF16, kind="ExternalInput")
    d_blob32 = nc.dram_tensor("blob32", [P, B32W], F32, kind="ExternalInput")
    d_embn = nc.dram_tensor("embn", [256, D], BF16, kind="ExternalInput")
    d_oetf = nc.dram_tensor("oetf", [256, T], BF16, kind="ExternalInput")
    d_oet = nc.dram_tensor("oet", [256, TL], BF16, kind="ExternalInput")
    d_fw = nc.dram_tensor("fw", [D], F32, kind="ExternalInput")
    d_wq = nc.dram_tensor("wq", [L, D, D], BF16, kind="ExternalInput")
    d_wk = nc.dram_tensor("wk", [L, D, D], BF16, kind="ExternalInput")
    d_wv = nc.dram_tensor("wv", [L, D, D], BF16, kind="ExternalInput")
    d_wo = nc.dram_tensor("wo", [L, D, D], BF16, kind="ExternalInput")
    d_w1 = nc.dram_tensor("w1", [L, D, FF], BF16, kind="ExternalInput")
    d_w2 = nc.dram_tensor("w2", [L, D, FF], BF16, kind="ExternalInput")
    d_w3 = nc.dram_tensor("w3", [L, FF, D], BF16, kind="ExternalInput")

    d_y = nc.dram_tensor("y", [TL, D], F32, kind="ExternalOutput")
    dbg_out = {}

    def dbg(name, shape, dtype=F32):
        if name in debug:
            dbg_out[name] = nc.dram_tensor(
                "dbg_" + name, shape, dtype, kind="ExternalOutput"
            )
            return dbg_out[name]
        return None

    with tile.TileContext(nc) as tc:
        with (
            tc.tile_pool(name="state", bufs=1) as state,
            tc.tile_pool(name="aux", bufs=1) as aux,
            tc.tile_pool(name="wsm", bufs=3) as wsm,
            tc.tile_pool(name="wff", bufs=4) as wff,
            tc.tile_pool(name="w3p", bufs=1) as w3p,
            tc.tile_pool(name="stp", bufs=2) as stp,
            tc.tile_pool(name="tmp", bufs=2) as tmp,
            tc.tile_pool(name="tmq", bufs=2) as tmq,
            tc.tile_pool(name="psum", bufs=4, space="PSUM") as psum,
            tc.tile_pool(name="psum_st", bufs=2, space="PSUM") as psum_st,
            tc.tile_pool(name="dram", bufs=1, space="DRAM") as dram,
        ):
            # ---- persistent state ----
            x_sb = state.tile([P, NT, D], F32, tag="x")          # local residual
            hTf = state.tile([P, DC, T], BF16, tag="hTf")        # full row h, D-major
            hTl = state.tile([P, DC, TL], BF16, tag="hTl")       # local h, D-major
            hT2 = state.tile([P, DC, TL], BF16, tag="hT2")       # local ffn h
            qT = state.tile([P, DC, TL], BF16, tag="qT")
            kTf = state.tile([P, DC, T], BF16, tag="kTf")
            vf = state.tile([P, KT2, H, HD + 1], BF16, tag="vf")
            oT = state.tile([P, DC, 2 * P], BF16, tag="oT")
            o_sb = state.tile([P, NT, D], BF16, tag="o_sb")
            xn = state.tile([P, NT, D], BF16, tag="xn")
            eps_sb = state.tile([P, 1], F32, tag="eps")
            nc.vector.memset(eps_sb[:], EPS)

            # packed tables (blob16 DMA issued after the embed chunks below)
            blob16 = aux.tile([P, B16W], BF16, tag="blob16")
            blob32 = aux.tile([P, B32W], F32, tag="blob32")
            embn_sb = aux.tile([P, 2, D], BF16, tag="embn")
            nc.sync.dma_start(
                embn_sb[:], d_embn.rearrange("(c p) d -> p c d", p=P)
            )
            nc.sync.dma_start(blob32[:], d_blob32[:])
            embn = embn_sb[:]

            def _b16(w):
                _b16.o += w
                return blob16[:, _b16.o - w : _b16.o]

            _b16.o = 0
            cosr = _b16(KT2 * NF).rearrange("p (t f) -> p t f", f=NF)
            sinr = _b16(KT2 * NF).rearrange("p (t f) -> p t f", f=NF)
            cosl = _b16(NT * NF).rearrange("p (t f) -> p t f", f=NF)
            sinl = _b16(NT * NF).rearrange("p (t f) -> p t f", f=NF)
            og_e = _b16(NT * SEG).rearrange("p (t s) -> p t s", s=SEG)
            otg_e = _b16(SC * TL).rearrange("p (c t) -> p c t", t=TL)
            obind_e = _b16(TL)
            bsel_e = _b16(2)
            normt = blob32[:, :NT]
            icnt_e = blob32[:, NT : NT + SC]

            def _halves(ap_, off):
                """[P,512] AP -> [P, H, NF] AP picking the off-th 32-half of
                each 64-wide head block."""
                return bass.AP(
                    tensor=ap_.tensor, offset=ap_.offset + off * NF,
                    ap=[list(ap_.ap[0]), [2 * NF, H], [1, NF]],
                )

            def rms_bf(xe, out_tile, out_slice):
                """out = xe * rsqrt(mean(xe^2)+eps); xe bf16 sbuf (phase-A:
                all-DVE fast path)."""
                sq = tmp.tile([P, D], BF16, tag="sq")
                ssq = tmp.tile([P, 1], F32, tag="ssq")
                nc.vector.tensor_tensor_reduce(
                    sq[:], xe[:], xe[:], 1.0, 0.0, ALU.mult, ALU.add, ssq[:]
                )
                nc.scalar.activation(
                    ssq[:], ssq[:], ACT.Sqrt, bias=eps_sb[:], scale=1.0 / D
                )
                nc.vector.reciprocal(ssq[:], ssq[:])
                nc.vector.tensor_scalar_mul(out_tile[out_slice], xe[:], ssq[:])

            def rms_f32(xsrc, out_tile, out_slice):
                """out = xsrc * rsqrt(mean^2+eps); xsrc f32 sbuf."""
                sq = tmp.tile([P, D], BF16, tag="sq")
                ssq = tmp.tile([P, 1], F32, tag="ssq")
                nc.vector.scalar_tensor_tensor(
                    sq[:], xsrc, 1.0, xsrc, ALU.mult, ALU.mult,
                    accum_out=ssq[:],
                )
                nc.scalar.activation(
                    ssq[:], ssq[:], ACT.Sqrt, bias=eps_sb[:], scale=1.0 / D
                )
                nc.vector.reciprocal(ssq[:], ssq[:])
                nc.vector.tensor_scalar_mul(out_tile[out_slice], xsrc, ssq[:])

            def rope_tok(ps, ct, st_, out):
                """token-major rope: out[p, d] over 512 cols.
                ps: [P,512] psum f32 (pre-rope proj); ct/st_: [P,NF] table
                slices; out: [P,512] bf16 sbuf tile. Act stages psum->bf16 so
                DVE runs in fast SBUF mode."""
                pp = tmq.tile([P, D], BF16, tag="pp", bufs=3)
                nc.scalar.copy(pp[:], ps)
                pp3 = pp[:].rearrange("p (a b) -> p a b", b=NF)
                crep = _rep_ap(ct, 2 * H)
                srep = _rep_ap(st_, H)
                t1 = tmq.tile([P, 2 * H, NF], BF16, tag="t1")
                ua = tmq.tile([P, H, NF], BF16, tag="ua")
                ub = tmq.tile([P, H, NF], BF16, tag="ub")
                nc.vector.tensor_tensor(t1[:], pp3, crep, ALU.mult)
                nc.vector.tensor_tensor(ua[:], _halves(pp[:], 1), srep, ALU.mult)
                nc.vector.tensor_tensor(ub[:], _halves(pp[:], 0), srep, ALU.mult)
                nc.vector.tensor_tensor(
                    _halves(out, 0), _halves(t1[:], 0), ua[:], ALU.subtract
                )
                nc.vector.tensor_tensor(
                    _halves(out, 1), _halves(t1[:], 1), ub[:], ALU.add
                )

            # ================= embedding =================
            # The embedding rows come from a 256-entry table, so rmsnorm(x0)
            # is a host-side table transform: h0 = onehot @ embn (normalized
            # table). No on-device norm at all here.
            # pass 1: full row (absolute order) -> h0 -> hTf
            for tg in range(4):
                oc = wff.tile([P, 2, 4 * P], BF16, tag="wffc")
                nc.gpsimd.dma_start(
                    oc[:],
                    d_oetf.rearrange("(c p) t -> p c t", p=P)[
                        :, :, tg * 4 * P : (tg + 1) * 4 * P
                    ],
                )
                for ti in range(4):
                    t = tg * 4 + ti
                    ps = psum.tile([P, 512], F32, tag="mm")
                    for kc in range(2):
                        nc.tensor.matmul(
                            ps[:],
                            oc[:, kc, ti * P : (ti + 1) * P],
                            embn[:, kc, :],
                            start=(kc == 0),
                            stop=(kc == 1),
                        )
                    h_t = tmq.tile([P, D], BF16, tag="ht", bufs=3)
                    nc.scalar.copy(h_t[:], ps[:])
                    nc.sync.dma_start_transpose(
                        hTf[:, :, t * P : (t + 1) * P], h_t[:]
                    )
            # layer-0 attention weights + tables: issued after the embed
            # chunks so they don't delay the first matmuls, but early enough
            # to land by phase A
            wqkv0 = []
            for d_w in (d_wq, d_wk, d_wv):
                w_sb = wsm.tile([P, DC, D], BF16, tag="wsm")
                nc.sync.dma_start(w_sb[:], d_w[0].rearrange("(c p) n -> p c n", p=P))
                wqkv0.append(w_sb)
            nc.sync.dma_start(blob16[:], d_blob16[:])

            # pass 2: local -> x_sb (raw) and hTl (normalized)
            for tg in range(2):
                oc = wff.tile([P, 2, 4 * P], BF16, tag="wffc")
                nc.gpsimd.dma_start(
                    oc[:],
                    d_oet.rearrange("(c p) t -> p c t", p=P)[
                        :, :, tg * 4 * P : (tg + 1) * 4 * P
                    ],
                )
                for ti in range(4):
                    t = tg * 4 + ti
                    ps2 = psum.tile([P, 512], F32, tag="mm")
                    for kc in range(2):
                        nc.tensor.matmul(
                            ps2[:],
                            oc[:, kc, ti * P : (ti + 1) * P],
                            embn[:, kc, :],
                            start=(kc == 0),
                            stop=(kc == 1),
                        )
                    # x = h * ||x||_rms (per-token scale from the table)
                    nc.vector.tensor_scalar_mul(
                        x_sb[:, t, :], ps2[:], normt[:, t : t + 1]
                    )
                    h_t = tmq.tile([P, D], BF16, tag="ht", bufs=3)
                    nc.scalar.copy(h_t[:], ps2[:])
                    nc.sync.dma_start_transpose(
                        hTl[:, :, t * P : (t + 1) * P], h_t[:]
                    )

            # boundary-exchange buffers (used by the last layer's tile-7 item)
            bseg_in = dram.tile([D], F32, tag="bseg_in")
            bseg_out = dram.tile([2, D], F32, tag="bseg_out")
            bg = tmp.tile([P, D], F32, tag="bg", bufs=1)
            nc.vector.memset(bg[:], 0.0)

            # ================= layers =================
            for l in range(L):
                if l == 0:
                    wq_sb, wk_sb, wv_sb = wqkv0
                else:
                    wq_sb = wsm.tile([P, DC, D], BF16, tag="wsm")
                    wk_sb = wsm.tile([P, DC, D], BF16, tag="wsm")
                    wv_sb = wsm.tile([P, DC, D], BF16, tag="wsm")
                    nc.sync.dma_start(
                        wq_sb[:], d_wq[l].rearrange("(c p) n -> p c n", p=P)
                    )
                    nc.sync.dma_start(
                        wk_sb[:], d_wk[l].rearrange("(c p) n -> p c n", p=P)
                    )
                    nc.sync.dma_start(
                        wv_sb[:], d_wv[l].rearrange("(c p) n -> p c n", p=P)
                    )

                # ---- q proj + rope (local) ----
                for t in range(NT):
                    ps = psum.tile([P, 512], F32, tag="mm")
                    for kc in range(DC):
                        nc.tensor.matmul(
                            ps[:],
                            hTl[:, kc, t * P : (t + 1) * P],
                            wq_sb[:, kc, :],
                            start=(kc == 0),
                            stop=(kc == DC - 1),
                        )
                    q_t = tmq.tile([P, D], BF16, tag="ht", bufs=3)
                    rope_tok(ps[:], cosl[:, t, :], sinl[:, t, :], q_t[:])
                    nc.sync.dma_start_transpose(qT[:, :, t * P : (t + 1) * P], q_t[:])

                # ---- k proj + rope and v (full row) ----
                # order: absolute tiles in exchange-arrival order
                korder = [0, 1, 8, 9, 2, 3, 10, 11, 4, 5, 12, 13, 6, 7, 14, 15]
                for t in korder:
                    ps = psum.tile([P, 512], F32, tag="mm")
                    for kc in range(DC):
                        nc.tensor.matmul(
                            ps[:],
                            hTf[:, kc, t * P : (t + 1) * P],
                            wk_sb[:, kc, :],
                            start=(kc == 0),
                            stop=(kc == DC - 1),
                        )
                    k_t = tmq.tile([P, D], BF16, tag="ht", bufs=3)
                    rope_tok(ps[:], cosr[:, t, :], sinr[:, t, :], k_t[:])
                    nc.sync.dma_start_transpose(kTf[:, :, t * P : (t + 1) * P], k_t[:])
                    ps2 = psum.tile([P, 512], F32, tag="mm")
                    for kc in range(DC):
                        nc.tensor.matmul(
                            ps2[:],
                            hTf[:, kc, t * P : (t + 1) * P],
                            wv_sb[:, kc, :],
                            start=(kc == 0),
                            stop=(kc == DC - 1),
                        )
                    nc.scalar.copy(
                        vf[:, t, :, :HD], ps2[:].rearrange("p (h d) -> p h d", h=H)
                    )
                    nc.vector.memset(vf[:, t, :, HD], 1.0)

                wo_sb = wsm.tile([P, DC, D], BF16, tag="wsm")
                nc.scalar.dma_start(wo_sb[:], d_wo[l].rearrange("(c p) n -> p c n", p=P))
                w3_sb = w3p.tile([P, FFC, D], BF16, tag="w3")
                # chunked + on the Act hwdge queue: keeps both the SP
                # queue and the DMA engine free for the embed transposes
                for c4 in range(4):
                    nc.scalar.dma_start(
                        w3_sb[:, c4 * 4 : (c4 + 1) * 4, :],
                        d_w3[l][c4 * 4 * P : (c4 + 1) * 4 * P].rearrange(
                            "(c p) n -> p c n", p=P
                        ),
                    )



                # ---- attention with software-pipelined FFN ----
                # FFN of q-block j is chopped into work items issued between
                # the head iterations of q-block j+1's attention, so the PE
                # chews FFN matmuls while the scalar engine runs exp.
                last = l == L - 1

                def attn(qb, h):
                    qsl = np.s_[qb * QBW : (qb + 1) * QBW]
                    po = (h % 2) * HD
                    hc = h // 2
                    st_sb = stp.tile([P, KT2, QBW], BF16, tag="st")
                    for k4 in (0, 2, 1, 3):
                        pst = psum_st.tile([P, 4, QBW], F32, tag="st")
                        for j in range(4):
                            kt = k4 * 4 + j
                            nc.tensor.matmul(
                                pst[:, j, :],
                                kTf[po : po + HD, hc, kt * P : (kt + 1) * P],
                                qT[po : po + HD, hc, qsl],
                                start=True,
                                stop=True,
                            )
                        nc.scalar.activation(
                            st_sb[:, k4 * 4 : (k4 + 1) * 4, :],
                            pst[:],
                            ACT.Exp,
                            scale=1.0 / math.sqrt(HD),
                        )
                    avkt = [0, 1, 2, 3, 8, 9, 10, 11, 4, 5, 6, 7, 12, 13, 14, 15]
                    for qt in range(QBW // P):
                        tix = qb * (QBW // P) + qt
                        pav = psum.tile([P, 512], F32, tag="mm")
                        for j, kt in enumerate(avkt):
                            nc.tensor.matmul(
                                pav[:, : HD + 1],
                                st_sb[:, kt, qt * P : (qt + 1) * P],
                                vf[:, kt, h, :],
                                start=(j == 0),
                                stop=(j == KT2 - 1),
                            )
                        rcp = tmp.tile([P, 1], F32, tag="rcp")
                        nc.vector.reciprocal(rcp[:], pav[:, HD : HD + 1])
                        nc.vector.tensor_scalar_mul(
                            o_sb[:, tix, h * HD : (h + 1) * HD],
                            pav[:, :HD],
                            rcp[:],
                        )

                def ffn_items(qb):
                    """Work items for q-block qb's wo+FFN (2 tiles)."""
                    tiles = [2 * qb, 2 * qb + 1]
                    if last and qb == 3:
                        tiles = [7, 6]
                    t0 = 2 * qb
                    nsl = np.s_[t0 * P : (t0 + 2) * P]
                    h12 = state.tile([P, FFC, 2, P], BF16, tag="h12")
                    items = []

                    def wo_item(tix):
                        loc = tix - t0
                        nc.sync.dma_start_transpose(
                            oT[:, :, loc * P : (loc + 1) * P], o_sb[:, tix, :]
                        )
                        ps = psum.tile([P, 512], F32, tag="mm")
                        for kc in range(DC):
                            nc.tensor.matmul(
                                ps[:],
                                oT[:, kc, loc * P : (loc + 1) * P],
                                wo_sb[:, kc, :],
                                start=(kc == 0),
                                stop=(kc == DC - 1),
                            )
                        nc.vector.tensor_add(
                            x_sb[:, tix, :], x_sb[:, tix, :], ps[:]
                        )
                        h2_t = tmq.tile([P, D], BF16, tag="ht", bufs=3)
                        rms_f32(x_sb[:, tix, :], h2_t, np.s_[:])
                        nc.sync.dma_start_transpose(
                            hT2[:, :, tix * P : (tix + 1) * P], h2_t[:]
                        )

                    def w12_item(mc2):
                        msl = np.s_[:, mc2 * 2 * P : (mc2 + 1) * 2 * P]
                        w1c = wff.tile([P, DC, 2 * P], BF16, tag="wffc")
                        w2c = wff.tile([P, DC, 2 * P], BF16, tag="wffc")
                        nc.sync.dma_start(
                            w1c[:], d_w1[l][msl].rearrange("(c p) n -> p c n", p=P)
                        )
                        nc.sync.dma_start(
                            w2c[:], d_w2[l][msl].rearrange("(c p) n -> p c n", p=P)
                        )
                        for mi in range(2):
                            mc = mc2 * 2 + mi
                            p1 = psum.tile([P, 512], F32, tag="mm")
                            p20 = psum_st.tile([P, 4, QBW], F32, tag="st", name="p20")
                            p2 = p20.rearrange("p a b -> p (a b)")[:, : 2 * P]
                            for kc in range(DC):
                                nc.tensor.matmul(
                                    p1[:, : 2 * P],
                                    w1c[:, kc, mi * P : (mi + 1) * P],
                                    hT2[:, kc, nsl],
                                    start=(kc == 0),
                                    stop=(kc == DC - 1),
                                )
                            for kc in range(DC):
                                nc.tensor.matmul(
                                    p2,
                                    w2c[:, kc, mi * P : (mi + 1) * P],
                                    hT2[:, kc, nsl],
                                    start=(kc == 0),
                                    stop=(kc == DC - 1),
                                )
                            sl = tmp.tile([P, 512], BF16, tag="sq")
                            hv = h12[:, mc, :, :].rearrange("p a b -> p (a b)")
                            if for_sim:
                                nc.scalar.activation(
                                    sl[:, : 2 * P], p1[:, : 2 * P], ACT.Sigmoid
                                )
                                u = tmp.tile([P, 512], BF16, tag="sq")
                                nc.vector.tensor_mul(
                                    u[:, : 2 * P], p1[:, : 2 * P], sl[:, : 2 * P]
                                )
                                nc.vector.tensor_mul(hv, p2, u[:, : 2 * P])
                            else:
                                nc.scalar.activation(
                                    sl[:, : 2 * P], p1[:, : 2 * P], ACT.Silu
                                )
                                nc.vector.tensor_mul(hv, p2, sl[:, : 2 * P])

                    def w3_item(tix):
                        tloc = tix - t0
                        ps = psum.tile([P, 512], F32, tag="mm")
                        for kc in range(FFC):
                            nc.tensor.matmul(
                                ps[:],
                                h12[:, kc, tloc, :],
                                w3_sb[:, kc, :],
                                start=(kc == 0),
                                stop=(kc == FFC - 1),
                            )
                        nc.vector.tensor_add(
                            x_sb[:, tix, :], x_sb[:, tix, :], ps[:]
                        )
                        if not last:
                            # next-layer h for this tile -> hTl (exchange)
                            h1_t = tmq.tile([P, D], BF16, tag="ht", bufs=3)
                            rms_f32(x_sb[:, tix, :], h1_t, np.s_[:])
                            nc.sync.dma_start_transpose(
                                hTl[:, :, tix * P : (tix + 1) * P], h1_t[:]
                            )
                        else:
                            # final-norm this tile now; on the boundary tile
                            # also fire the boundary-partial pair exchange
                            rms_f32(x_sb[:, tix, :], xn, np.s_[:, tix, :])
                            if tix == 7:
                                pex = psum.tile([P, 512], F32, tag="mm")
                                nc.tensor.matmul(
                                    pex[:1, :], bsel_e[:, 0:1], xn[:, 0, :],
                                    start=True, stop=False,
                                )
                                nc.tensor.matmul(
                                    pex[:1, :], bsel_e[:, 1:2], xn[:, 7, :],
                                    start=False, stop=True,
                                )
                                bpart = tmp.tile([1, D], F32, tag="bx", bufs=1)
                                nc.vector.tensor_copy(bpart[:], pex[:1, :])
                                nc.gpsimd.dma_start(
                                    bseg_in[:].rearrange("(a d) -> a d", a=1),
                                    bpart[:],
                                )
                                nc.gpsimd.collective_compute(
                                    "AllGather",
                                    ALU.bypass,
                                    replica_groups=[[0, 1], [2, 3], [4, 5], [6, 7]],
                                    ins=[bseg_in[:].opt()],
                                    outs=[bseg_out[:].opt()],
                                )
                                nc.gpsimd.dma_start(bg[:2, :], bseg_out[:])

                    def cc_item():
                        # pair-exchange this slice of next-layer h. All DMAs
                        # ride the Pool (SWDGE) queue so they never block the
                        # SP queue's weight streams behind a collective wait.
                        nb = D * 2 * P
                        bh_in = dram.tile([nb], BF16, tag=f"bh_in{qb}")
                        bh_out = dram.tile([2 * nb], BF16, tag=f"bh_out{qb}")
                        nc.gpsimd.dma_start(
                            bh_in[:].rearrange("(p c x) -> p c x", p=P, c=DC),
                            hTl[:, :, nsl],
                        )
                        nc.gpsimd.collective_compute(
                            "AllGather",
                            ALU.bypass,
                            replica_groups=[[0, 1], [2, 3], [4, 5], [6, 7]],
                            ins=[bh_in[:].opt()],
                            outs=[bh_out[:].opt()],
                        )
                        for r in range(2):
                            nc.gpsimd.dma_start(
                                hTf[
                                    :, :, r * TL + t0 * P : r * TL + (t0 + 2) * P
                                ],
                                bh_out[r * nb :][:nb].rearrange(
                                    "(p c x) -> p c x", p=P, c=DC
                                ),
                            )

                    for tix in tiles:
                        items.append(lambda tix=tix: wo_item(tix))
                    for mc2 in range(FFC // 2):
                        items.append(lambda mc2=mc2: w12_item(mc2))
                    for tix in tiles:
                        items.append(lambda tix=tix: w3_item(tix))
                    if not last:
                        items.append(cc_item)
                    return items

                # layer 0 compresses the FFN spread into the first 6 head
                # slots so each pair-exchange launches ~2 slots earlier
                slots = 7
                pend = []
                for qb in range(NQB):
                    for h in range(H):
                        attn(qb, h)
                        if pend:
                            if h == 0:
                                n_take = 2  # just wo+rms; hT2 not ready yet
                            elif h < slots:
                                n_take = -(-len(pend) // (slots - h))
                            else:
                                n_take = len(pend)
                            for it in pend[:n_take]:
                                it()
                            pend = pend[n_take:]
                    pend = ffn_items(qb)
                for it in pend:
                    it()

            # ================= segment pooling =================
            # (final norm + boundary exchange already ran inside the last
            # layer's w3 items)
            if apply_fw:
                fw_bc = aux.tile([P, D], F32, tag="fw_bc")
                nc.sync.dma_start(
                    fw_bc[:],
                    bass.AP(tensor=d_fw, offset=0, ap=[[0, P], [1, D]]),
                )

            # segment sums -> means
            segsum_bf = aux.tile([P, SC, D], BF16, tag="segsum_bf")
            for mc in range(SC):
                ps = psum.tile([P, 512], F32, tag="mm")
                for kt in range(NT):
                    nc.tensor.matmul(
                        ps[:],
                        og_e[:, kt, mc * P : (mc + 1) * P],
                        xn[:, kt, :],
                        start=(kt == 0),
                        stop=(kt == NT - 1),
                    )
                nc.scalar.copy(segsum_bf[:, mc, :], ps[:])

            segmean = aux.tile([P, SC, D], BF16, tag="segmean")
            for mc in range(SC):
                nc.vector.tensor_scalar_mul(
                    segmean[:, mc, :], segsum_bf[:, mc, :], icnt_e[:, mc : mc + 1]
                )
            bgfix = tmp.tile([P, D], BF16, tag="bgf")
            nc.vector.tensor_copy(bgfix[:], bg[:])

            # out = 0.5*xn + scatter(segmean) [+ obind.T @ partner_partial]
            # Partner boundary correction only ever lands in tiles {0, NT-1}
            # (host-asserted), and runs in separate psum groups at the very
            # end so the main scatter never waits on the boundary collective.
            def scat(t):
                ps = psum.tile([P, 512], F32, tag="mm")
                for kc in range(SC):
                    nc.tensor.matmul(
                        ps[:],
                        otg_e[:, kc, t * P : (t + 1) * P],
                        segmean[:, kc, :],
                        start=(kc == 0),
                        stop=(kc == SC - 1),
                    )
                out_t = tmp.tile([P, D], F32, tag="out", bufs=2)
                nc.vector.scalar_tensor_tensor(
                    out_t[:], xn[:, t, :], ALPHA, ps[:], ALU.mult, ALU.add
                )
                return out_t

            def emit(t, out_t):
                if apply_fw:
                    nc.vector.tensor_mul(out_t[:], out_t[:], fw_bc[:])
                nc.sync.dma_start(
                    d_y.rearrange("(t p) d -> t p d", p=P)[t], out_t[:]
                )

            for t in range(1, NT - 1):
                emit(t, scat(t))
            bouts = {t: scat(t) for t in (0, NT - 1)}
            for t in (0, NT - 1):
                pc = psum_st.tile([P, 4, QBW], F32, tag="st")
                pcv = pc.rearrange("p a b -> p (a b)")[:, :512]
                nc.tensor.matmul(
                    pcv,
                    obind_e[:, t * P : (t + 1) * P],
                    bgfix[:],
                    start=True,
                    stop=True,
                )
                out_t = bouts[t]
                nc.vector.tensor_add(out_t[:], out_t[:], pcv)
                emit(t, out_t)

            # ---- debug taps ----
            if "x0" in debug:
                nc.sync.dma_start(
                    dbg_out["x0"].rearrange("(t p) d -> t p d", p=P)[:], x_sb[:]
                )

    strip_transpose_cc_waits(nc)
    if patch:
        split_multiwait_drains(nc)
    return nc


# ----------------------------------------------------------------------------
# host side
# ----------------------------------------------------------------------------


def _to_bf16(a):
    return np.asarray(a, dtype=np.float32).astype(ml_dtypes.bfloat16)


def host_prep(inputs):
    tokens = np.clip(np.asarray(inputs["tokens"]), 0, 255).astype(np.int64)
    emb = np.asarray(inputs["embed_table"], np.float32)
    attn_w = np.asarray(inputs["attn_norm_w"], np.float32)
    ffn_w = np.asarray(inputs["ffn_norm_w"], np.float32)
    fin_w = np.asarray(inputs["final_norm_w"], np.float32)
    wq = np.asarray(inputs["wq"], np.float32) * attn_w[:, :, None]
    wk = np.asarray(inputs["wk"], np.float32) * attn_w[:, :, None]
    wv = np.asarray(inputs["wv"], np.float32) * attn_w[:, :, None]
    wo = np.asarray(inputs["wo"], np.float32)
    w1 = np.asarray(inputs["w1"], np.float32) * ffn_w[:, :, None]
    w2 = np.asarray(inputs["w2"], np.float32) * ffn_w[:, :, None]
    w3 = np.asarray(inputs["w3"], np.float32)

    norm_tab = np.sqrt((emb.astype(np.float64) ** 2).mean(-1) + EPS)
    embn = (emb / norm_tab[:, None].astype(np.float32)).astype(np.float32)

    shared = {
        "fw": fin_w,
        "wq": _to_bf16(wq),
        "wk": _to_bf16(wk),
        "wv": _to_bf16(wv),
        "wo": _to_bf16(wo),
        "w1": _to_bf16(w1),
        "w2": _to_bf16(w2),
        "w3": _to_bf16(w3),
    }

    def _pc(a, chunk):
        """[C*P, X] -> [P, C*X] per-partition pack (c-major columns)."""
        a = np.asarray(a)
        c = a.shape[0] // P
        return a.reshape(c, P, -1).transpose(1, 0, 2).reshape(P, -1)

    embn_pk_rows = embn  # [256, D], program rearranges (c p) d -> p c d

    # rope tables, token-major: table[p, t, j] = f((t*128+p) * inv[j])
    inv = 1.0 / (10000.0 ** (np.arange(0, HD, 2, dtype=np.float64) / HD))  # (NF,)
    posf = np.arange(T, dtype=np.float64)
    ang = posf[:, None] * inv[None, :]                 # (T, NF)
    cosT_full = np.cos(ang).reshape(KT2, P, NF).transpose(1, 0, 2)  # (P, KT2, NF)
    sinT_full = np.sin(ang).reshape(KT2, P, NF).transpose(1, 0, 2)

    in_maps = []
    for c in range(N_CORES):
        b, half = c // 2, c % 2
        tok_full = tokens[b]
        tok = tok_full[half * TL : (half + 1) * TL]

        oetf = np.zeros((256, T), np.float32)
        oetf[tok_full, np.arange(T)] = 1.0
        oet = np.zeros((256, TL), np.float32)
        oet[tok, np.arange(TL)] = 1.0

        cosl = cosT_full[:, half * NT : (half + 1) * NT, :]
        sinl = sinT_full[:, half * NT : (half + 1) * NT, :]

        # segments
        is_sep = SEP_TABLE[tok_full]
        seg = np.cumsum(is_sep.astype(np.int64))  # inclusive, full row
        cnt = np.bincount(seg, minlength=seg[-1] + 1).astype(np.float64)
        ids = seg[half * TL : (half + 1) * TL]
        base = ids[0]
        loc = ids - base
        S_loc = int(loc[-1]) + 1
        assert S_loc <= SEG, f"too many segments {S_loc}"
        og = np.zeros((TL, SEG), np.float32)
        og[np.arange(TL), loc] = 1.0
        icnt = np.ones(SEG, np.float64)
        icnt[:S_loc] = 0.5 / np.maximum(cnt[base : base + S_loc], 1.0)
        bsel = np.zeros((P, 2), np.float32)
        obind = np.zeros((P, TL), np.float32)
        if seg[TL - 1] == seg[TL]:  # a segment spans the half boundary
            s_b = int(seg[TL]) if half == 1 else int(seg[TL - 1])
            mask = ids == s_b
            # boundary-segment tokens must lie in tile 0 or tile NT-1
            inner = mask.copy()
            inner[:P] = False
            inner[(NT - 1) * P :] = False
            assert not inner.any(), "boundary segment spans interior tiles"
            bsel[:, 0] = mask[:P]
            bsel[:, 1] = mask[(NT - 1) * P :]
            obind[1 - half, :] = mask * (0.5 / max(cnt[s_b], 1.0))

        blob16 = np.concatenate(
            [
                cosT_full.reshape(P, -1),                         # [P, KT2*NF]
                sinT_full.reshape(P, -1),
                cosl.reshape(P, -1),
                sinl.reshape(P, -1),
                _pc(og, NT),                                      # [P, NT*SEG]
                _pc(og.T.copy(), SC),                             # [P, SC*TL]
                obind,                                            # [P, TL]
                bsel,                                             # [P, 2]
            ],
            axis=1,
        )
        normt = norm_tab[tok].reshape(NT, P).T                    # [P, NT]
        blob32 = np.concatenate(
            [normt, icnt.reshape(SC, P).T], axis=1
        ).astype(np.float32)

        in_maps.append(
            dict(
                shared,
                oetf=_to_bf16(oetf),
                oet=_to_bf16(oet),
                embn=_to_bf16(embn_pk_rows),
                blob16=_to_bf16(blob16),
                blob32=blob32,
            )
        )
    return in_maps


class Runner:
    """Compile once; keep inputs device-resident; re-upload only changed data."""

    def __init__(self, nc):
        import jax
        import jax.numpy as jnp
        from jax.experimental.shard_map import shard_map
        from jax.sharding import Mesh, PartitionSpec
        import concourse.mybir as mybir_
        from concourse import bass2jax

        bass2jax.install_neuronx_cc_hook()
        self.jax = jax
        self.nc = nc
        in_names, out_names, out_avals, zero_outs = [], [], [], []
        for alloc in nc.m.functions[0].allocations:
            if not isinstance(mybir_.MemoryLocationSet, type) or not isinstance(
                alloc, mybir_.MemoryLocationSet
            ):
                continue
            name = alloc.memorylocations[0].name
            if alloc.kind == "ExternalInput":
                if nc.partition_id_tensor is None or name != nc.partition_id_tensor.name:
                    in_names.append(name)
            elif alloc.kind == "ExternalOutput":
                shape = tuple(alloc.tensor_shape)
                dtype = mybir_.dt.np(alloc.dtype)
                out_names.append(name)
                out_avals.append(jax.core.ShapedArray(shape, dtype))
                zero_outs.append(np.zeros(shape, dtype))
        self.n_params = len(in_names)
        self.in_names = list(in_names)
        self.out_names = out_names
        all_in_names = in_names + out_names
        partition_name = nc.partition_id_tensor.name if nc.partition_id_tensor else None
        if partition_name is not None:
            all_in_names = all_in_names + [partition_name]

        def _body(*args):
            operands = list(args)
            if partition_name is not None:
                operands.append(bass2jax.partition_id_tensor())
            outs = bass2jax._bass_exec_p.bind(
                *operands,
                out_avals=tuple(out_avals),
                in_names=tuple(all_in_names),
                out_names=tuple(out_names),
                lowering_input_output_aliases=(),
                sim_require_finite=True,
                sim_require_nnan=True,
                nc=nc,
            )
            return tuple(outs)

        devices = jax.devices()[:N_CORES]
        mesh = Mesh(np.asarray(devices), ("core",))
        n_in = self.n_params + len(out_names)
        self.sharded = jax.jit(
            shard_map(
                _body,
                mesh=mesh,
                in_specs=(PartitionSpec("core"),) * n_in,
                out_specs=(PartitionSpec("core"),) * len(out_names),
                check_rep=False,
            ),
            keep_unused=True,
        )
        self.mesh = mesh
        self.zero_outs = zero_outs
        self._dev_zero = None
        self._cache_np = {}
        self._cache_dev = {}

    def _put(self, name, arrs):
        """Concat per-core numpy arrays and put sharded on device (cached)."""
        import jax
        from jax.sharding import NamedSharding, PartitionSpec

        cached = self._cache_np.get(name)
        if cached is not None and all(
            a is b or (a.shape == b.shape and np.array_equal(a, b))
            for a, b in zip(cached, arrs)
        ):
            return self._cache_dev[name]
        glob = np.concatenate([np.asarray(a) for a in arrs], axis=0)
        dev = jax.device_put(glob, NamedSharding(self.mesh, PartitionSpec("core")))
        self._cache_np[name] = [np.asarray(a) for a in arrs]
        self._cache_dev[name] = dev
        return dev

    def __call__(self, in_maps):
        import jax
        from jax.sharding import NamedSharding, PartitionSpec

        args = [
            self._put(name, [m[name] for m in in_maps]) for name in self.in_names
        ]
        if self._dev_zero is None:
            self._dev_zero = [
                jax.device_put(
                    np.zeros((N_CORES * z.shape[0], *z.shape[1:]), z.dtype),
                    NamedSharding(self.mesh, PartitionSpec("core")),
                )
                for z in self.zero_outs
            ]
        outs = self.sharded(*args, *self._dev_zero)
        outs = [np.asarray(o) for o in outs]
        return {
            name: outs[i].reshape(N_CORES, *self.zero_outs[i].shape)
            for i, name in enumerate(self.out_names)
        }


_RUNNER = None
_RUNNER_FLAGS = None


def _get_runner(apply_fw=False):
    global _RUNNER, _RUNNER_FLAGS
    if _RUNNER is None or _RUNNER_FLAGS != (apply_fw,):
        nc = build_program(apply_fw=apply_fw)
        _RUNNER = Runner(nc)
        _RUNNER_FLAGS = (apply_fw,)
    return _RUNNER


def kernel(**inputs):
    apply_fw = not np.allclose(np.asarray(inputs["final_norm_w"]), 1.0)
    runner = _get_runner(apply_fw=apply_fw)
    in_maps = host_prep(inputs)
    res = runner(in_maps)
    y = res["y"]
    out = np.zeros((B, T, D), np.float32)
    for c in range(N_CORES):
        b, half = c // 2, c % 2
        out[b, half * TL : (half + 1) * TL, :] = y[c]
    return out
